# revision 7
# baseline (speedup 1.0000x reference)
"""Trainium2 Bass kernel for the gnn_message_passing block (8 NeuronCores).

Strategy (per core c, owning 512 global rows r = c*512..(c+1)*512):
  - Host rotates x_node/x_edge rows by -r0 so the owned rows sit first on
    every core (SPMD: one program, per-core data).
  - Associativity: mat @ (x @ W.T) == (mat @ x) @ W.T, so the five big
    N x N aggregations (adj@h shared by modules 0/4, four proj@k inputs)
    are computed ONCE per core as row-blocks (mat[r].T streamed from DRAM
    as the matmul moving operand; h/e tiles as stationary), producing
    feature-major outputs that feed the 512x512 projections directly.
  - rmsnorm weight vectors and the 1/sqrt(D) score scale are folded into
    the projection weights host-side; on-chip rmsnorm is the pure
    x * rsqrt(mean(x^2)+eps) form.
  - Per-node 8-head SDPA runs on DVE with broadcast-AP multiplies and
    segmented reduces; exp on ACT.
  - FFNs: feature-major matmuls, gelu(+bias) fused on ACT, bias2 added
    via a K=1 ones matmul into PSUM.
All matmuls use float32r (full-rate fp32 on TRN2 when moving dim >= 256).
"""
import numpy as np

N = 4096
E = 512
H = 8
D = 64
FF = 2048
P = 128
NCORES = 8
RPC = N // NCORES  # 512 rows per core
NT = N // P        # 32 tiles over all nodes
LT = RPC // P      # 4 local tiles
EPS = float(np.finfo(np.float32).eps)

_PROGRAM_CACHE = {}


def _split_big_waits(nc, mybir):
    """walrus in this toolchain rejects >4 sem waits on one instruction; the
    TileContext final drain can exceed that. Split extras into preceding
    EventSemaphore instructions on the same engine."""
    for f in nc.m.functions:
        for bb in f.blocks:
            insts = list(bb.instructions)
            out = []
            changed = False
            for inst in insts:
                si = inst.sync_info
                waits = list(si.on_wait) if si and si.on_wait else []
                cap = 2 if isinstance(inst, mybir.InstEventSemaphore) else 1
                if len(waits) > cap:
                    extra, keep = waits[:-cap], waits[-cap:]
                    for ci in range(0, len(extra), 2):
                        ev = mybir.InstEventSemaphore(name=f"{inst.name}-evw{ci}")
                        ev.engine = inst.engine
                        ev.sync_info = mybir.SyncInfo(on_wait=extra[ci:ci + 2],
                                                      on_update=[])
                        out.append(ev)
                    si.on_wait = keep
                    changed = True
                out.append(inst)
            if changed:
                bb.instructions[:] = out


def _build_program():
    import concourse.bass as bass
    import concourse.tile as tile
    from concourse import mybir
    from concourse.masks import make_identity
    from contextlib import ExitStack

    f32 = mybir.dt.float32
    f32r = mybir.dt.float32r
    AF = mybir.ActivationFunctionType
    OP = mybir.AluOpType
    AX = mybir.AxisListType

    def bc(t, dims):
        """Broadcast/strided view of tile AP t with free dims = (step, count)."""
        return bass.AP(tensor=t.tensor, offset=t.offset,
                       ap=[list(t.ap[0])] + [[s, c] for (s, c) in dims])

    nc = bass.Bass()

    xn_d = nc.declare_dram_parameter("xn", [N, E], f32, isOutput=False)
    xe_d = nc.declare_dram_parameter("xe", [N, E], f32, isOutput=False)
    mat_d = [nc.declare_dram_parameter(f"mat{i}", [N, RPC], f32, isOutput=False)
             for i in range(5)]
    wq_d = nc.declare_dram_parameter("wqT", [H, E, E], f32, isOutput=False)
    wk_d = nc.declare_dram_parameter("wkT", [H, E, E], f32, isOutput=False)
    wv_d = nc.declare_dram_parameter("wvT", [H, E, E], f32, isOutput=False)
    w1h_d = nc.declare_dram_parameter("w1hT", [E, FF], f32, isOutput=False)
    w2h_d = nc.declare_dram_parameter("w2hT", [FF, E], f32, isOutput=False)
    w1e_d = nc.declare_dram_parameter("w1eT", [E, FF], f32, isOutput=False)
    w2e_d = nc.declare_dram_parameter("w2eT", [FF, E], f32, isOutput=False)
    b1h_d = nc.declare_dram_parameter("b1h", [FF], f32, isOutput=False)
    b2h_d = nc.declare_dram_parameter("b2h", [E], f32, isOutput=False)
    b1e_d = nc.declare_dram_parameter("b1e", [FF], f32, isOutput=False)
    b2e_d = nc.declare_dram_parameter("b2e", [E], f32, isOutput=False)
    outh_d = nc.declare_dram_parameter("outh", [RPC, E], f32, isOutput=True)
    oute_d = nc.declare_dram_parameter("oute", [RPC, E], f32, isOutput=True)

    with tile.TileContext(nc) as tc, ExitStack() as ctx:
        consts = ctx.enter_context(tc.tile_pool(name="consts", bufs=1))
        ident = consts.tile([P, P], f32)
        make_identity(nc, ident)
        ones1f = consts.tile([1, P], f32)
        nc.gpsimd.memset(ones1f, 1.0)
        ones1 = consts.tile([1, P], f32r)
        nc.scalar.copy(ones1[:], ones1f[:])
        eps_t = consts.tile([P, 1], f32)
        nc.vector.memset(eps_t, EPS)
        b1h_t = consts.tile([P, FF // P], f32)
        nc.gpsimd.dma_start(out=b1h_t, in_=b1h_d[:].rearrange("(c p) -> p c", p=P))
        b1e_t = consts.tile([P, FF // P], f32)
        nc.gpsimd.dma_start(out=b1e_t, in_=b1e_d[:].rearrange("(c p) -> p c", p=P))
        b2h_t = consts.tile([1, E], f32r)
        nc.gpsimd.dma_start(out=b2h_t, in_=b2h_d[:].rearrange("(a e) -> a e", a=1))
        b2e_t = consts.tile([1, E], f32r)
        nc.gpsimd.dma_start(out=b2e_t, in_=b2e_d[:].rearrange("(a e) -> a e", a=1))

        # long-lived pools (agg outputs, local transposed h/e, branch accum)
        aggp = ctx.enter_context(tc.tile_pool(name="agg", bufs=1))
        locp = ctx.enter_context(tc.tile_pool(name="loc", bufs=1))
        attp = ctx.enter_context(tc.tile_pool(name="att", bufs=1))
        statp = ctx.enter_context(tc.tile_pool(name="stat", bufs=4))

        aggT = [[None] * 4 for _ in range(5)]   # feature-major [128f, 512n] tiles
        hTl = [locp.tile([P, RPC], f32r, tag=f"hTl{fc}", name=f"hTl{fc}") for fc in range(4)]
        eTl = [locp.tile([P, RPC], f32r, tag=f"eTl{fc}", name=f"eTl{fc}") for fc in range(4)]
        xatt_h = [attp.tile([P, E], f32, tag=f"xh{t}", name=f"xh{t}") for t in range(LT)]
        xatt_e = [attp.tile([P, E], f32, tag=f"xe{t}", name=f"xe{t}") for t in range(LT)]

        def rmsnorm_tile(dst, src_dram_slice, xpool):
            """dst[128, 512] = pure rmsnorm of one node-major tile."""
            xt = xpool.tile([P, E], f32, tag="xin")
            nc.gpsimd.dma_start(out=xt, in_=src_dram_slice)
            stats = statp.tile([P, 6], f32, tag="bst")
            nc.vector.bn_stats(out=stats[:], in_=xt[:])
            mv = statp.tile([P, 2], f32, tag="bag")
            nc.vector.bn_aggr(out=mv[:], in_=stats[:])
            m2 = statp.tile([P, 1], f32, tag="m2")
            nc.vector.tensor_tensor(out=m2[:], in0=mv[:, 0:1], in1=mv[:, 0:1],
                                    op=OP.mult)
            nc.vector.tensor_tensor(out=m2[:], in0=m2[:], in1=mv[:, 1:2], op=OP.add)
            sq = statp.tile([P, 1], f32, tag="sq")
            nc.scalar.activation(out=sq[:], in_=m2[:], func=AF.Sqrt,
                                 bias=eps_t[:], scale=1.0)
            rs = statp.tile([P, 1], f32, tag="rs")
            nc.vector.reciprocal(out=rs[:], in_=sq[:])
            nc.scalar.activation(out=dst[:], in_=xt[:], func=AF.Copy, scale=rs[:])

        def transpose_local(srcs, dstT, psp):
            """srcs: 4 node-major [128, 512] tiles (local block);
            dstT: 4 feature-major [128, 512] tiles."""
            for fc in range(4):
                ps = psp.tile([P, RPC], f32, tag="trps", bufs=2)
                for t in range(4):
                    nc.tensor.transpose(ps[:, t * P:(t + 1) * P],
                                        srcs[t][:, fc * P:(fc + 1) * P].bitcast(f32), ident[:])
                nc.scalar.copy(dstT[fc][:], ps[:])

        def aggregate(mi, lhs_tiles, psp, mpool):
            """aggT[mi][b] = feature-major block of (mat_mi @ x)."""
            pss = [psp.tile([P, E], f32, tag=f"agps{b}", name=f"agps{b}") for b in range(4)]
            for t in range(NT):
                mt = mpool.tile([P, RPC], f32r, tag="matt")
                nc.gpsimd.dma_start(out=mt, in_=mat_d[mi][t * P:(t + 1) * P, :])
                for b in range(4):
                    nc.tensor.matmul(pss[b][:],
                                     lhsT=lhs_tiles[t][:, b * P:(b + 1) * P],
                                     rhs=mt[:],
                                     start=(t == 0), stop=(t == NT - 1))
            for b in range(4):
                at = aggp.tile([P, E], f32r, tag=f"agg{mi}_{b}", name=f"agg{mi}_{b}")
                nc.scalar.copy(at[:], pss[b][:])
                aggT[mi][b] = at

        # ---- phase 1a/2a: h, its local transpose, h-based aggregations ----
        with tc.tile_pool(name="hfull", bufs=1) as hfp, \
             tc.tile_pool(name="xin_a", bufs=4) as xpool_a, \
             tc.tile_pool(name="ps_a", bufs=1, space="PSUM") as psp_a, \
             tc.tile_pool(name="mat_a", bufs=6) as mpool_a:
            h_sb = [hfp.tile([P, E], f32r, tag=f"h{t}", name=f"hsb{t}") for t in range(NT)]
            for t in range(NT):
                rmsnorm_tile(h_sb[t], xn_d[t * P:(t + 1) * P, :], xpool_a)
            transpose_local(h_sb[:4], hTl, psp_a)
            for mi in (0, 1, 2):
                aggregate(mi, h_sb, psp_a, mpool_a)

        # ---- phase 1b/2b: e, its local transpose, e-based aggregations ----
        with tc.tile_pool(name="efull", bufs=1) as efp, \
             tc.tile_pool(name="xin_b", bufs=4) as xpool_b, \
             tc.tile_pool(name="ps_b", bufs=1, space="PSUM") as psp_b, \
             tc.tile_pool(name="mat_b", bufs=6) as mpool_b:
            e_sb = [efp.tile([P, E], f32r, tag=f"e{t}", name=f"esb{t}") for t in range(NT)]
            for t in range(NT):
                rmsnorm_tile(e_sb[t], xe_d[t * P:(t + 1) * P, :], xpool_b)
            transpose_local(e_sb[:4], eTl, psp_b)
            for mi in (3, 4):
                aggregate(mi, e_sb, psp_b, mpool_b)

        # ---- phase 3: the 8 attention modules ----
        # module -> (q_inT, k_inT); v_inT is always hTl
        def m_inputs(m):
            qsrc = {0: aggT[0], 1: eTl, 2: eTl, 3: hTl,
                    4: aggT[0], 5: eTl, 6: eTl, 7: hTl}[m]
            ksrc = {0: hTl, 1: eTl, 2: aggT[1], 3: aggT[3],
                    4: hTl, 5: eTl, 6: aggT[2], 7: aggT[4]}[m]
            return qsrc, ksrc

        MORDER = [0, 4, 2, 3, 6, 7, 1, 5]

        with tc.tile_pool(name="wts", bufs=2) as wpool, \
             tc.tile_pool(name="qkv", bufs=1) as qkvp, \
             tc.tile_pool(name="sdtmp", bufs=2) as tmpp, \
             tc.tile_pool(name="sdsm", bufs=4) as smp, \
             tc.tile_pool(name="ps_c", bufs=1, space="PSUM") as psp_c:
            for m in MORDER:
                qsrc, ksrc = m_inputs(m)
                branch_att = xatt_h if m < 4 else xatt_e
                first = (m % 4 == 0) if False else (m in (0, 4))
                # branch accumulation starts at module 0 (h) / 4 (e); given
                # MORDER, module 0 and 4 are processed first in each branch.
                wq_t, wk_t, wv_t = [], [], []
                for fc in range(4):
                    for (lst, dram, nm) in ((wq_t, wq_d, "wq"), (wk_t, wk_d, "wk"),
                                            (wv_t, wv_d, "wv")):
                        wt = wpool.tile([P, E], f32r, tag=f"{nm}{fc}", name=f"w_{nm}{fc}")
                        nc.gpsimd.dma_start(
                            out=wt, in_=dram[m, fc * P:(fc + 1) * P, :])
                        lst.append(wt)

                q_sb, k_sb, v_sb = [], [], []
                for (srcT, w_t, lst, nm) in ((qsrc, wq_t, q_sb, "q"),
                                             (ksrc, wk_t, k_sb, "k"),
                                             (hTl, wv_t, v_sb, "v")):
                    for b in range(LT):
                        ps = psp_c.tile([P, E], f32, tag="projps", bufs=4)
                        for fc in range(4):
                            nc.tensor.matmul(
                                ps[:],
                                lhsT=srcT[fc][:, b * P:(b + 1) * P],
                                rhs=w_t[fc][:],
                                start=(fc == 0), stop=(fc == 3))
                        dt = qkvp.tile([P, E], f32r, tag=f"{nm}{b}", name=f"qkv_{nm}{b}")
                        nc.scalar.copy(dt[:], ps[:])
                        lst.append(dt)

                for t in range(LT):
                    q_t = q_sb[t][:].bitcast(f32)
                    k_t = k_sb[t][:].bitcast(f32)
                    v_t = v_sb[t][:].bitcast(f32)
                    # scores s[p, h, g] = sum_d q[p,h,d] * k[p,g,d]
                    tmp = tmpp.tile([P, H * H * D], f32, tag="sdpa")
                    nc.vector.tensor_tensor(
                        out=bc(tmp, [(512, 8), (64, 8), (1, 64)]),
                        in0=bc(q_t, [(64, 8), (0, 8), (1, 64)]),
                        in1=bc(k_t, [(0, 8), (64, 8), (1, 64)]),
                        op=OP.mult)
                    s_t = smp.tile([P, H * H], f32, tag="s")
                    nc.vector.tensor_reduce(
                        out=s_t[:], in_=tmp.rearrange("p (a d) -> p a d", d=D),
                        axis=AX.X, op=OP.add)
                    ex_t = smp.tile([P, H * H], f32, tag="ex")
                    nc.scalar.activation(out=ex_t[:], in_=s_t[:], func=AF.Exp)
                    den = smp.tile([P, H], f32, tag="den")
                    nc.vector.tensor_reduce(
                        out=den[:], in_=ex_t.rearrange("p (h g) -> p h g", g=H),
                        axis=AX.X, op=OP.add)
                    rden = smp.tile([P, H], f32, tag="rden")
                    nc.vector.reciprocal(out=rden[:], in_=den[:])
                    a_t = smp.tile([P, H * H], f32, tag="a")
                    nc.vector.tensor_tensor(
                        out=bc(a_t, [(8, 8), (1, 8)]),
                        in0=bc(ex_t, [(8, 8), (1, 8)]),
                        in1=bc(rden, [(1, 8), (0, 8)]),
                        op=OP.mult)
                    # attn out[p, h, d] = sum_g a[p,h,g] * v[p,g,d]
                    tmp2 = tmpp.tile([P, H * H * D], f32, tag="sdpa")
                    nc.vector.tensor_tensor(
                        out=bc(tmp2, [(512, 8), (8, 64), (1, 8)]),
                        in0=bc(a_t, [(8, 8), (0, 64), (1, 8)]),
                        in1=bc(v_t, [(0, 8), (1, 64), (64, 8)]),
                        op=OP.mult)
                    if m in (0, 4):
                        nc.vector.tensor_reduce(
                            out=branch_att[t][:],
                            in_=tmp2.rearrange("p (a g) -> p a g", g=H),
                            axis=AX.X, op=OP.add)
                    else:
                        rt = smp.tile([P, E], f32, tag="avred")
                        nc.vector.tensor_reduce(
                            out=rt[:], in_=tmp2.rearrange("p (a g) -> p a g", g=H),
                            axis=AX.X, op=OP.add)
                        nc.vector.tensor_tensor(out=branch_att[t][:],
                                                in0=branch_att[t][:], in1=rt[:],
                                                op=OP.add)

        # ---- phase 4: rms2 + FFN per branch ----
        def ffn(branch_att, w1_dram, w2_dram, b1_t, b2_t, out_dram):
            with tc.tile_pool(name="ffn_sb", bufs=1) as fsb, \
                 tc.tile_pool(name="ffn_xn", bufs=1) as fxn, \
                 tc.tile_pool(name="ffn_ps", bufs=1, space="PSUM") as fps, \
                 tc.tile_pool(name="ffn_out", bufs=4) as fout:
                # rms2 (pure; gamma folded into w1) then transpose to
                # feature-major xnT
                xn_tiles = []
                for t in range(LT):
                    xt = fxn.tile([P, E], f32, tag=f"fx{t}", name=f"fx{t}")
                    stats = statp.tile([P, 6], f32, tag="bst")
                    nc.vector.bn_stats(out=stats[:], in_=branch_att[t][:])
                    mv = statp.tile([P, 2], f32, tag="bag")
                    nc.vector.bn_aggr(out=mv[:], in_=stats[:])
                    m2 = statp.tile([P, 1], f32, tag="m2")
                    nc.vector.tensor_tensor(out=m2[:], in0=mv[:, 0:1],
                                            in1=mv[:, 0:1], op=OP.mult)
                    nc.vector.tensor_tensor(out=m2[:], in0=m2[:], in1=mv[:, 1:2],
                                            op=OP.add)
                    sq = statp.tile([P, 1], f32, tag="sq")
                    nc.scalar.activation(out=sq[:], in_=m2[:], func=AF.Sqrt,
                                         bias=eps_t[:], scale=1.0)
                    rs = statp.tile([P, 1], f32, tag="rs")
                    nc.vector.reciprocal(out=rs[:], in_=sq[:])
                    nc.scalar.activation(out=xt[:], in_=branch_att[t][:],
                                         func=AF.Copy, scale=rs[:])
                    xn_tiles.append(xt)
                xnT = []
                for fc in range(4):
                    ps = fps.tile([P, RPC], f32, tag="ftr", bufs=2)
                    for t in range(4):
                        nc.tensor.transpose(ps[:, t * P:(t + 1) * P],
                                            xn_tiles[t][:, fc * P:(fc + 1) * P],
                                            ident[:])
                    xt = fxn.tile([P, RPC], f32r, tag=f"fxT{fc}", name=f"fxT{fc}")
                    nc.scalar.copy(xt[:], ps[:])
                    xnT.append(xt)
                # FFN1 + gelu(+b1)
                w1_t = [fsb.tile([P, FF], f32r, tag=f"w1_{fc}", name=f"w1_{fc}") for fc in range(4)]
                for fc in range(4):
                    nc.gpsimd.dma_start(out=w1_t[fc],
                                        in_=w1_dram[fc * P:(fc + 1) * P, :])
                g1 = []
                for ffb in range(FF // P):
                    ps = fps.tile([P, RPC], f32, tag="fps1", bufs=4)
                    for fc in range(4):
                        nc.tensor.matmul(
                            ps[:],
                            lhsT=w1_t[fc][:, ffb * P:(ffb + 1) * P],
                            rhs=xnT[fc][:],
                            start=(fc == 0), stop=(fc == 3))
                    gt = fsb.tile([P, RPC], f32r, tag=f"g1_{ffb}", name=f"g1_{ffb}")
                    nc.scalar.activation(out=gt[:], in_=ps[:], func=AF.Gelu,
                                         bias=b1_t[:, ffb:ffb + 1], scale=1.0)
                    g1.append(gt)
                # FFN2 + b2 via K=1 ones matmul
                w2_t = [fsb.tile([P, E], f32r, tag=f"w2_{fc}", name=f"w2_{fc}") for fc in range(FF // P)]
                for ffc in range(FF // P):
                    nc.gpsimd.dma_start(out=w2_t[ffc],
                                        in_=w2_dram[ffc * P:(ffc + 1) * P, :])
                for b in range(LT):
                    ps = fps.tile([P, E], f32, tag="fps2", bufs=2)
                    for ffc in range(FF // P):
                        nc.tensor.matmul(
                            ps[:],
                            lhsT=g1[ffc][:, b * P:(b + 1) * P],
                            rhs=w2_t[ffc][:],
                            start=(ffc == 0), stop=False)
                    nc.tensor.matmul(ps[:], lhsT=ones1[:],
                                     rhs=b2_t[:],
                                     start=False, stop=True)
                    ot = fout.tile([P, E], f32, tag="fo")
                    nc.scalar.copy(ot[:], ps[:])
                    nc.gpsimd.dma_start(out=out_dram[b * P:(b + 1) * P, :], in_=ot)

        ffn(xatt_h, w1h_d, w2h_d, b1h_t, b2h_t, outh_d)
        ffn(xatt_e, w1e_d, w2e_d, b1e_t, b2e_t, oute_d)

    _split_big_waits(nc, mybir)
    return nc


def _get_program():
    if "nc" not in _PROGRAM_CACHE:
        _PROGRAM_CACHE["nc"] = _build_program()
    return _PROGRAM_CACHE["nc"]


def _prep_inputs(x_node, x_edge, adj, Wq, Wk, Wv,
                 proj_he_h, proj_eh_h, proj_he_e, proj_eh_e,
                 rms1_h, rms1_e, rms2_h,
                 w1_h, b1_h, w2_h, b2_h, w1_e, b1_e, w2_e, b2_e):
    """Per-core input dicts. Weight folding + row rotation happen here."""
    f = np.float32
    # q_in source rms weight per module; k_in source; v always h.
    wsrc_q = [rms1_h, rms1_e, rms1_e, rms1_h, rms1_h, rms1_e, rms1_e, rms1_h]
    wsrc_k = [rms1_h, rms1_e, rms1_h, rms1_e, rms1_h, rms1_e, rms1_h, rms1_e]
    wqT = np.stack([(Wq[m].T * wsrc_q[m][:, None]) * 0.125 for m in range(H)])
    wkT = np.stack([Wk[m].T * wsrc_k[m][:, None] for m in range(H)])
    wvT = np.stack([Wv[m].T * rms1_h[:, None] for m in range(H)])
    w1hT = np.ascontiguousarray((w1_h * rms2_h[None, :]).T, dtype=f)
    w1eT = np.ascontiguousarray((w1_e * rms2_h[None, :]).T, dtype=f)
    w2hT = np.ascontiguousarray(w2_h.T, dtype=f)
    w2eT = np.ascontiguousarray(w2_e.T, dtype=f)
    mats = [adj, proj_eh_h, proj_eh_e, proj_he_h, proj_he_e]

    shared = dict(wqT=np.ascontiguousarray(wqT, f), wkT=np.ascontiguousarray(wkT, f),
                  wvT=np.ascontiguousarray(wvT, f),
                  w1hT=w1hT, w2hT=w2hT, w1eT=w1eT, w2eT=w2eT,
                  b1h=b1_h.astype(f), b2h=b2_h.astype(f),
                  b1e=b1_e.astype(f), b2e=b2_e.astype(f))
    in_maps = []
    for c in range(NCORES):
        r0 = c * RPC
        m = dict(shared)
        m["xn"] = np.ascontiguousarray(np.roll(x_node, -r0, axis=0), f)
        m["xe"] = np.ascontiguousarray(np.roll(x_edge, -r0, axis=0), f)
        for i, mat in enumerate(mats):
            mt = np.ascontiguousarray(mat[r0:r0 + RPC].T, f)  # [N, RPC]
            m[f"mat{i}"] = np.ascontiguousarray(np.roll(mt, -r0, axis=0))
        in_maps.append(m)
    return in_maps


def kernel(**inputs):
    from concourse.bass_utils import run_bass_kernel_spmd
    nc = _get_program()
    in_maps = _prep_inputs(**{k: np.asarray(v) for k, v in inputs.items()})
    res = run_bass_kernel_spmd(nc, in_maps, list(range(NCORES))).results
    x_h = np.concatenate([res[c]["outh"] for c in range(NCORES)], axis=0)
    x_e = np.concatenate([res[c]["oute"] for c in range(NCORES)], axis=0)
    return (x_h, x_e)
